# revision 1
# baseline (speedup 1.0000x reference)
"""Trainium2 Bass kernel for nn_CrossAttention (gnn_message_passing).

Reference computation (per batch b, point n):
  nb[c,n,o]  = sum_f neighbors[c,n,f] * W_two[o,f] + b_two[o]
  q[n,e]     = sum_c pcd[n,c] Wq[e,c]
  k[e,n,o]   = sum_c Wk[e,c] nb[c,n,o]
  v[e,n,o]   = sum_c Wv[e,c] nb[c,n,o]
  scores     = sum_d q[n,(h,d)] k[(h,d),n,o] / sqrt(8)
  attn       = softmax_o(scores)
  out[(h,d),n] = sum_o attn[h,n,o] v[(h,d),n,o]

Sharding: data-parallel over (b, n-block): 8 cores, each takes 256 points.

Device pipeline per core (n=256 points, c=64, f=512, o=256, h=8, d=8):
  S1: nb[(n,c), o] via fp32r matmuls, lhsT = host-transposed neighbors chunks
      [f=128, cn=128] (stationary), rhs = W_two^T chunks [f=128, o=256].
  S2: per (n, o-half): out[o-half=128, 128] = nb_n[c=64, o-half]^T @
      [Wv^T | qc_{8n-block}] -> v_T (cols 0-63) + scores_T (col 64+8j+h).
      qc[h,n,c] = sum_d q[n,(h,d)] Wk[(h,d),c]/sqrt(8) precomputed on host
      (19 MFLOP, 0.4% of total; pure reparametrization of q@k).
  softmax: scores stay [o-partitions, free]; exp on ACT (no max-subtract:
      |scores| ~ 0.05 for this problem's scales); Z via ones-matmul.
  S4: xc[0:64, h] = sum_o v_T[o,e] exp[o,h], xc[64:72, h] = Z[h] via
      ones-columns packed into the v tile. Normalize at the end.
"""

import math
import ml_dtypes
import numpy as np
from contextlib import ExitStack

import concourse.bass as bass
import concourse.tile as tile
from concourse import bacc, mybir
from concourse.bass_utils import run_bass_kernel_spmd

F32R = mybir.dt.float32r
F32 = mybir.dt.float32
BF16 = mybir.dt.bfloat16

NCORES = 8
B, N, C, LF = 2, 1024, 64, 256
F2 = 2 * LF          # 512 neighbor features
O = LF               # 256 attention keys per point
H, D = 8, 8          # heads, depth
NP = (B * N) // NCORES  # 256 points per core
G = NP // 8          # 32 groups of 8 points

_BUILD_CACHE = {}
STAGE = 4  # debug: 1=S1 only, 2=+S2, 3=+exp, 4=full
FEATURES = {"s2mm", "vevac", "stevac", "memset", "exp", "s4"}


def build_nc(with_bias: bool, repeat: int = 1, g_mod: int = G):
    """Build the per-core Bass module.

    g_mod: number of groups present in the nbt input (the g-loop reads
    nbt[g % g_mod]); g_mod == G for real runs, smaller for timing builds.
    repeat: device-side repetition count (For_i) for timing.
    """
    key = (with_bias, repeat, g_mod, STAGE, tuple(sorted(FEATURES)))
    if key in _BUILD_CACHE:
        return _BUILD_CACHE[key]

    nc = bacc.Bacc("TRN2", target_bir_lowering=False, debug=False)
    # DRAM I/O
    nbt_d = nc.dram_tensor("nbt", [g_mod, 4, 128, 512], F32R, kind="ExternalInput").ap()
    w2t_d = nc.dram_tensor("w2t", [4, 128, 256], F32R, kind="ExternalInput").ap()
    r2_d = nc.dram_tensor("r2", [G, 128, 128], BF16, kind="ExternalInput").ap()
    b2_d = nc.dram_tensor("b2", [1, 256], F32R, kind="ExternalInput").ap()
    xcout_d = nc.dram_tensor("xcout", [72, NP, 8], F32, kind="ExternalOutput").ap()

    with tile.TileContext(nc) as tc, ExitStack() as ctx:
        singles = ctx.enter_context(tc.tile_pool(name="singles", bufs=1))
        slabs = ctx.enter_context(tc.tile_pool(name="slabs", bufs=3))
        nbs = ctx.enter_context(tc.tile_pool(name="nbs", bufs=3))
        vs = ctx.enter_context(tc.tile_pool(name="vs", bufs=3))
        sts = ctx.enter_context(tc.tile_pool(name="sts", bufs=3))
        ps_nb = ctx.enter_context(tc.tile_pool(name="ps_nb", bufs=1, space="PSUM"))
        ps_vq = ctx.enter_context(tc.tile_pool(name="ps_vq", bufs=2, space="PSUM"))
        ps_xc = ctx.enter_context(tc.tile_pool(name="ps_xc", bufs=2, space="PSUM"))

        # one-time loads
        w2t = singles.tile([128, 4, 256], F32R)
        nc.sync.dma_start(out=w2t, in_=w2t_d.rearrange("a p c -> p a c"))
        r2 = singles.tile([128, G, 128], BF16)
        nc.sync.dma_start(out=r2, in_=r2_d.rearrange("g p c -> p g c"))
        if with_bias:
            b2 = singles.tile([1, 256], F32R)
            nc.sync.dma_start(out=b2, in_=b2_d)
            ones1 = singles.tile([1, 128], F32R)
            nc.vector.memset(ones1.bitcast(F32), 1.0)

        xc_pool = ctx.enter_context(tc.tile_pool(name="xc_full", bufs=1))
        xc_holder = {}

        def body(_i=None):
            xc_full = xc_pool.tile([128, NP, 8], F32, tag="xcf")
            xc_holder["t"] = xc_full
            nc.gpsimd.memset(xc_full, 0.0)
            for g in range(G):
                gi = g % g_mod
                # ---- S1: nb[(n,c), o] for the 8 points of this group ----
                slab = slabs.tile([128, 4, 512], F32R, tag="slab")
                nc.gpsimd.dma_start(out=slab, in_=nbt_d[gi].rearrange("a p c -> p a c"))
                nb_ps = ps_nb.tile([128, 1024], F32, tag="nbps")
                for t in range(4):
                    for ci in range(4):
                        nc.tensor.matmul(
                            nb_ps[:, 256 * t : 256 * t + 256],
                            slab[:, ci, 128 * t : 128 * t + 128],
                            w2t[:, ci, :],
                            start=(ci == 0),
                            stop=(ci == 3) and not with_bias,
                        )
                    if with_bias:
                        nc.tensor.matmul(
                            nb_ps[:, 256 * t : 256 * t + 256],
                            ones1,
                            b2,
                            start=False,
                            stop=True,
                        )
                nb_sb = nbs.tile([128, 4, 256], BF16, tag="nb")
                nc.vector.tensor_copy(nb_sb[:, 0:2, :], nb_ps[:, 0:512])
                nc.scalar.copy(nb_sb[:, 2:4, :], nb_ps[:, 512:1024])
                # odd-n copies shifted to partition base 0 (HW rejects K=64
                # matmuls with operands at partition base 64)
                nb_od = nbs.tile([64, 4, 256], BF16, tag="nbod")
                nc.vector.tensor_copy(nb_od[:, 0:2, :], nb_ps[64:128, 0:512])
                nc.scalar.copy(nb_od[:, 2:4, :], nb_ps[64:128, 512:1024])

                # ---- S2: v_T + scores_T per (n, o-half) ----
                if STAGE < 2:
                    continue
                exp_sb = sts.tile([128, 16, 8], BF16, tag="exp")
                v_g = vs.tile([128, 16, 128], BF16, tag="v")
                # ones cols 64-72, zeros 72-128 for the S4 stationary tiles
                if "memset" in FEATURES:
                    nc.gpsimd.memset(v_g[:, :, 64:72], 1.0)
                    nc.gpsimd.memset(v_g[:, :, 72:128], 0.0)
                for a in range(2):
                    vq = ps_vq.tile([128, 1024], F32, tag="vq")
                    for m in range(4):
                        nl = 4 * a + m       # n within group (0..7)
                        t = nl // 2          # nb subtile
                        par = nl % 2
                        src = nb_sb if par == 0 else nb_od
                        for half in range(2):
                            nc.tensor.matmul(
                                vq[:, 128 * (2 * m + half) : 128 * (2 * m + half) + 128],
                                src[0:64, t, 128 * half : 128 * half + 128],
                                r2[0:64, g, :],
                                start=True,
                                stop=True,
                            )
                    # v_T evac (cols 0..64 of each slot) on ACT, cast bf16
                    vq3 = vq.rearrange("p (s x) -> p s x", s=8)
                    if "vevac" in FEATURES:
                        nc.scalar.copy(v_g[:, 8 * a : 8 * a + 8, 0:64], vq3[:, :, 0:64])
                    # scores_T evac: col 64+8*(4a+m)+h of slot (2m+half)
                    st_in = bass.AP(
                        tensor=vq.tensor,
                        offset=vq.offset + 64 + 32 * a,
                        ap=[vq.ap[0], [264, 4], [128, 2], [1, 8]],
                    )
                    st_sb = sts.tile([128, 2, 4, 2, 8], F32, tag="st")
                    if "stevac" in FEATURES:
                        nc.vector.tensor_copy(st_sb[:, a], st_in)
                    # exp on ACT -> bf16 (no max subtraction; |scores| << 1)
                    if STAGE < 3:
                        continue
                    nc.scalar.activation(
                        out=exp_sb[:, 8 * a : 8 * a + 8, :].rearrange("p s x -> p (s x)"),
                        in_=st_sb[:, a].rearrange("p a b c -> p (a b c)"),
                        func=mybir.ActivationFunctionType.Exp,
                        scale=1.0,
                    )

                # ---- S4: xc[e|Z, h] per n, accumulate o-halves ----
                if STAGE < 4:
                    continue
                xc_ps = ps_xc.tile([128, 64], F32, tag="xc")
                for nl in range(8):
                    a, m = nl // 4, nl % 4
                    for half in range(2):
                        slot = 8 * a + 2 * m + half
                        nc.tensor.matmul(
                            xc_ps[:, 8 * nl : 8 * nl + 8],
                            v_g[:, slot, :],
                            exp_sb[:, slot, :],
                            start=(half == 0),
                            stop=(half == 1),
                        )
                nc.vector.tensor_copy(xc_full[:, 8 * g : 8 * g + 8, :], xc_ps)

        if repeat > 1:
            with tc.For_i(0, repeat, 1):
                body()
        else:
            body()

        # ---- tail: ship raw xc (x rows 0-63, Z replicas rows 64-71) ----
        xc_full = xc_holder["t"]
        nc.sync.dma_start(out=xcout_d, in_=xc_full[0:72])

    nc.compile()
    _BUILD_CACHE[key] = nc
    return nc


def host_prep(pcd, neighbors, W_two, b_two, Wq, Wk, Wv):
    """Per-core input maps (host-side layout transforms + q/qc fold)."""
    scale = 1.0 / math.sqrt(D)
    # q[b,n,e] then qc[b,h,n,c] = sum_d q[b,n,(h,d)] Wk[(h,d),c] * scale
    q = np.einsum("bnc,ec->bne", pcd, Wq).astype(np.float32)
    qc = np.einsum("bnhd,hdc->bhnc", q.reshape(B, N, H, D), Wk.reshape(H, D, C))
    qc = (qc * scale).astype(np.float32)

    w2t = np.ascontiguousarray(W_two.T.reshape(4, 128, O)).astype(np.float32)
    b2 = b_two.reshape(1, O).astype(np.float32)
    with_bias = bool(np.any(b_two))

    in_maps = []
    npb = N // (NCORES // B)  # points per core
    for core in range(NCORES):
        b = core // (NCORES // B)
        n0 = (core % (NCORES // B)) * npb
        nb = neighbors[b, :, n0 : n0 + npb, :]          # (c, np, f)
        # nbt[g, ci, fi, cn] with cn = (n within group)*64 + c
        nbt = np.transpose(nb, (2, 1, 0)).reshape(F2, G, 8 * C)   # (f, g, cn)
        nbt = np.transpose(nbt, (1, 0, 2)).reshape(G, 4, 128, 8 * C)
        nbt = np.ascontiguousarray(nbt).astype(np.float32)
        # r2[g, c(x2), col]: cols 0-63 = Wv^T, 64+8j+h = qc[h, 8g+j, c]
        r2 = np.zeros((G, 128, 128), np.float32)
        r2[:, 0:64, 0:64] = np.broadcast_to(Wv.T, (G, C, C))
        qc_core = qc[b, :, n0 : n0 + npb, :]             # (h, np, c)
        # [g, c, 8j+h]
        qjc = np.transpose(qc_core.reshape(H, G, 8, C), (1, 3, 2, 0)).reshape(G, C, 64)
        r2[:, 0:64, 64:128] = qjc
        r2[:, 64:128, :] = r2[:, 0:64, :]
        r2 = r2.astype(ml_dtypes.bfloat16)
        in_maps.append({"nbt": nbt, "w2t": w2t, "r2": r2, "b2": b2})
    return in_maps, with_bias


def kernel(pcd, neighbors, W_two, b_two, Wq, Wk, Wv):
    in_maps, with_bias = host_prep(pcd, neighbors, W_two, b_two, Wq, Wk, Wv)
    nc = build_nc(with_bias)
    res = run_bass_kernel_spmd(nc, in_maps, list(range(NCORES)))
    out = np.empty((B, C, N), np.float32)
    npb = N // (NCORES // B)
    hh = np.arange(C) // D  # head index per output channel
    for core in range(NCORES):
        b = core // (NCORES // B)
        n0 = (core % (NCORES // B)) * npb
        xc = res.results[core]["xcout"]          # [72, NP, 8]
        x = xc[np.arange(C), :, hh]              # [C, NP] numerator
        z = xc[64, :, hh]                        # [C, NP] denominator (Z replicas)
        out[b, :, n0 : n0 + npb] = x / z
    return out



# revision 3
# speedup vs baseline: 18.0645x; 18.0645x over previous
"""Trainium2 Bass kernel for nn_CrossAttention (gnn_message_passing).

Self-contained: hardcoded shapes B=2,N=1024,C=64,LF=256, 8 cores,
data-parallel over (batch, point-block); 256 points per core.

Same m-form data flow as kernel2, but each step processes 2 groups with
one 1.18MB DMA (alternating sync/gpsimd rings) and one set of 8 W2^T
stationaries streaming 1024+128 cols each — halving weight loads and
DMA fixed overheads per point.

PSUM budget (8 banks): psA = 2 tiles [128,1024]f32 (bufs=1) = 4 banks,
psW = [128,1024]f32 (bufs=2) = 4 banks with column map
  0:128 scores h0 | 128:256 scores h1 | 256+64s m-slice s(0..7)
  | 768:896 Z/b2e | 896:1024 mm.
"""

import math
import ml_dtypes
import numpy as np
from contextlib import ExitStack

import concourse.bass as bass
import concourse.tile as tile
from concourse import bacc, mybir
from concourse.bass_utils import run_bass_kernel_spmd

F32 = mybir.dt.float32
BF16 = mybir.dt.bfloat16

NCORES = 8
B, N, C, LF = 2, 1024, 64, 256
F2 = 2 * LF
O = LF
H, D = 8, 8
NP = (B * N) // NCORES
G = NP // 8          # 32 groups of 8 points
SG = G // 2          # 16 supergroups of 16 points

_BUILD_CACHE = {}
STAGE = 3  # 0=DMA only, 1=+S1 matmuls, 2=+evac/exp/m/Z, 3=full
DMAQ = "alt-gpsimd"  # sync | alt-scalar | alt-gpsimd
SLAB_BUFS = 4


def build_nc(repeat: int = 1, s_mod: int = SG):
    key = (repeat, s_mod, STAGE, DMAQ, SLAB_BUFS)
    if key in _BUILD_CACHE:
        return _BUILD_CACHE[key]

    nc = bacc.Bacc("TRN2", target_bir_lowering=False, debug=False)
    comb_d = nc.dram_tensor("comb", [s_mod, 128, 4, 1152], BF16, kind="ExternalInput").ap()
    w2t_d = nc.dram_tensor("w2t", [128, 4, 256], BF16, kind="ExternalInput").ap()
    wvt_d = nc.dram_tensor("wvt", [64, 64], BF16, kind="ExternalInput").ap()
    ob2_d = nc.dram_tensor("ob2", [128, 2, 2], BF16, kind="ExternalInput").ap()
    xcout_d = nc.dram_tensor("xcout", [66, G, 64], F32, kind="ExternalOutput").ap()

    with tile.TileContext(nc) as tc, ExitStack() as ctx:
        singles = ctx.enter_context(tc.tile_pool(name="singles", bufs=1))
        slabs = ctx.enter_context(tc.tile_pool(name="slabs", bufs=SLAB_BUFS))
        nbs = ctx.enter_context(tc.tile_pool(name="nbs", bufs=2))
        exps = ctx.enter_context(tc.tile_pool(name="exps", bufs=2))
        msts = ctx.enter_context(tc.tile_pool(name="msts", bufs=2))
        ps_a = ctx.enter_context(tc.tile_pool(name="ps_a", bufs=1, space="PSUM"))
        ps_w = ctx.enter_context(tc.tile_pool(name="ps_w", bufs=2, space="PSUM"))

        w2t = singles.tile([128, 4, 256], BF16)
        nc.sync.dma_start(out=w2t, in_=w2t_d)
        wvt = singles.tile([64, 64], BF16)
        nc.sync.dma_start(out=wvt, in_=wvt_d)
        ob2 = singles.tile([128, 2, 2], BF16)
        nc.sync.dma_start(out=ob2, in_=ob2_d)

        out_pool = ctx.enter_context(tc.tile_pool(name="out_full", bufs=1))
        out_holder = {}

        def body(_i=None):
            out_sb = out_pool.tile([66, G, 64], F32, tag="osb")
            out_holder["t"] = out_sb
            if STAGE < 3:
                nc.gpsimd.memset(out_sb, 0.0)
            for s in range(SG):
                si = s % s_mod
                slab = slabs.tile([128, 4, 1152], BF16, tag="slab")
                deng = nc.sync
                if s % 2 == 1:
                    if DMAQ == "alt-scalar":
                        deng = nc.scalar
                    elif DMAQ == "alt-gpsimd":
                        deng = nc.gpsimd
                deng.dma_start(out=slab, in_=comb_d[si])
                if STAGE < 1:
                    continue

                # ---- S1: nb_T + scores_T, stationary = w2t chunks ----
                psW = ps_w.tile([128, 1024], F32, tag="psW")
                psA00 = ps_a.tile([128, 512], F32, tag="psA00")
                psA01 = ps_a.tile([128, 512], F32, tag="psA01")
                psA10 = ps_a.tile([128, 512], F32, tag="psA10")
                psA11 = ps_a.tile([128, 512], F32, tag="psA11")
                psA = [[psA00, psA01], [psA10, psA11]]
                nbsb = nbs.tile([128, 2, 1024], BF16, tag="nb")
                exp_sb = exps.tile([128, 2, 128], BF16, tag="exp")
                for half in range(2):
                    for ci in range(4):
                        lw = w2t[:, ci, 128 * half : 128 * half + 128]
                        for k in range(2):
                            nc.tensor.matmul(
                                psA[half][k], lw, slab[:, ci, 512 * k : 512 * k + 512],
                                start=(ci == 0), stop=(ci == 3),
                            )
                        nc.tensor.matmul(
                            psW[:, 128 * half : 128 * half + 128],
                            lw, slab[:, ci, 1024:1152],
                            start=(ci == 0), stop=(ci == 3),
                        )
                    if STAGE < 2:
                        continue
                    # evac nb_T (bf16); split halves across DVE / ACT
                    if half == 0:
                        nc.vector.tensor_copy(nbsb[:, 0, 0:512], psA[0][0])
                        nc.vector.tensor_copy(nbsb[:, 0, 512:1024], psA[0][1])
                    else:
                        nc.scalar.copy(nbsb[:, 1, 0:512], psA[1][0])
                        nc.scalar.copy(nbsb[:, 1, 512:1024], psA[1][1])
                    # exp straight from PSUM on ACT
                    nc.scalar.activation(
                        out=exp_sb[:, half, :],
                        in_=psW[:, 128 * half : 128 * half + 128],
                        func=mybir.ActivationFunctionType.Exp,
                        scale=1.0,
                    )

                # ---- m[cn, (j,h)] + Z/b2e rows ----
                if STAGE < 2:
                    continue
                for t in range(8):
                    k = t // 4  # group within supergroup
                    for half in range(2):
                        nc.tensor.matmul(
                            psW[:, 256 + 64 * t : 320 + 64 * t],
                            nbsb[:, half, 128 * t : 128 * t + 128],
                            exp_sb[:, half, 64 * k : 64 * k + 64],
                            start=(half == 0), stop=(half == 1),
                        )
                for half in range(2):
                    nc.tensor.matmul(
                        psW[0:2, 768:896], ob2[:, half, :], exp_sb[:, half, :],
                        start=(half == 0), stop=(half == 1),
                    )

                # ---- diagonal gather m -> mst[c, (k,j,h)] ----
                if STAGE < 3:
                    continue
                mst = msts.tile([64, 128], BF16, tag="mst")
                for k in range(2):
                    in_ev = bass.AP(
                        tensor=psW.tensor, offset=psW.offset + 256 + 256 * k,
                        ap=[psW[0:64].ap[0], [80, 4], [1, 8]],
                    )
                    in_od = bass.AP(
                        tensor=psW.tensor,
                        offset=psW[64:128].offset + 256 + 256 * k + 8,
                        ap=[psW[64:128].ap[0], [80, 4], [1, 8]],
                    )
                    out_ev = bass.AP(
                        tensor=mst.tensor, offset=mst.offset + 64 * k,
                        ap=[mst.ap[0], [16, 4], [1, 8]],
                    )
                    out_od = bass.AP(
                        tensor=mst.tensor, offset=mst.offset + 64 * k + 8,
                        ap=[mst.ap[0], [16, 4], [1, 8]],
                    )
                    nc.vector.tensor_copy(out_ev, in_ev)
                    nc.vector.tensor_copy(out_od, in_od)

                # ---- mm[e, (k,j,h)] = Wv^T m ----
                nc.tensor.matmul(
                    psW[0:64, 896:1024], wvt, mst, start=True, stop=True,
                )
                nc.vector.tensor_copy(
                    out_sb[0:64, 2 * s : 2 * s + 2, :],
                    bass.AP(tensor=psW.tensor, offset=psW.offset + 896,
                            ap=[psW[0:64].ap[0], [64, 2], [1, 64]]),
                )
                nc.vector.tensor_copy(
                    out_sb[64:66, 2 * s : 2 * s + 2, :],
                    bass.AP(tensor=psW.tensor, offset=psW.offset + 768,
                            ap=[psW[0:2].ap[0], [64, 2], [1, 64]]),
                )

        if repeat > 1:
            with tc.For_i(0, repeat, 1):
                body()
        else:
            body()

        out_sb = out_holder["t"]
        nc.sync.dma_start(out=xcout_d, in_=out_sb)

    nc.compile()
    _BUILD_CACHE[key] = nc
    return nc


def host_prep(pcd, neighbors, W_two, b_two, Wq, Wk, Wv):
    scale = 1.0 / math.sqrt(D)
    pcd = np.asarray(pcd, np.float32)
    neighbors = np.asarray(neighbors, np.float32)
    W_two = np.asarray(W_two, np.float32)
    b_two = np.asarray(b_two, np.float32)
    Wq = np.asarray(Wq, np.float32)
    Wk = np.asarray(Wk, np.float32)
    Wv = np.asarray(Wv, np.float32)

    q = np.einsum("bnc,ec->bne", pcd, Wq).astype(np.float32)
    qc = np.einsum("bnhd,hdc->bhnc", q.reshape(B, N, H, D), Wk.reshape(H, D, C))
    qc = (qc * scale).astype(np.float32)

    if np.any(b_two):
        xstar = np.linalg.lstsq(W_two, b_two, rcond=None)[0]
    else:
        xstar = np.zeros((F2,), np.float32)

    w2t = np.ascontiguousarray(
        W_two.T.reshape(4, 128, O).transpose(1, 0, 2)).astype(ml_dtypes.bfloat16)
    wvt = np.ascontiguousarray(Wv.T).astype(ml_dtypes.bfloat16)
    ob2 = np.zeros((128, 2, 2), np.float32)
    ob2[:, :, 0] = 1.0
    ob2[:, 0, 1] = b_two[0:128]
    ob2[:, 1, 1] = b_two[128:256]
    ob2 = ob2.astype(ml_dtypes.bfloat16)
    wvs = Wv.sum(axis=1).astype(np.float32)

    in_maps = []
    npb = N // (NCORES // B)
    for core in range(NCORES):
        b = core // (NCORES // B)
        n0 = (core % (NCORES // B)) * npb
        nbr = neighbors[b, :, n0 : n0 + npb, :]          # (c, np, f)
        nbt = np.transpose(nbr, (2, 1, 0))               # (f, np, c)
        qc_core = qc[b, :, n0 : n0 + npb, :]             # (h, np, c)
        qn = np.matmul(qc_core.transpose(1, 0, 2), nbr.transpose(1, 0, 2))
        if np.any(b_two):
            qcs = qc_core.sum(axis=2).T
            qn = qn + qcs[:, :, None] * xstar[None, None, :]
        comb = np.empty((SG, 128, 4, 1152), np.float32)
        # nbt: comb[sg, fi, ci, 512*k + 64*j + c] = nbt[128*ci+fi, 16*sg+8*k+j, c]
        a = nbt.reshape(4, 128, SG, 2 * 8 * C)           # (ci, fi, sg, kcn)
        comb[:, :, :, 0:1024] = a.transpose(2, 1, 0, 3)
        # qnbr: comb[sg, fi, ci, 1024 + 64*k + 8*j + h]
        qb = qn.reshape(SG, 2, 8, H, 4, 128)             # (sg, k, j, h, ci, fi)
        comb[:, :, :, 1024:1152] = qb.transpose(0, 4, 5, 1, 2, 3).reshape(SG, 4, 128, 128).transpose(0, 2, 1, 3)
        comb = comb.astype(ml_dtypes.bfloat16)
        in_maps.append({"comb": comb, "w2t": w2t, "wvt": wvt, "ob2": ob2})
    return in_maps, wvs


def kernel(pcd, neighbors, W_two, b_two, Wq, Wk, Wv):
    in_maps, wvs = host_prep(pcd, neighbors, W_two, b_two, Wq, Wk, Wv)
    nc = build_nc()
    res = run_bass_kernel_spmd(nc, in_maps, list(range(NCORES)))
    out = np.empty((B, C, N), np.float32)
    npb = N // (NCORES // B)
    for core in range(NCORES):
        b = core // (NCORES // B)
        n0 = (core % (NCORES // B)) * npb
        xc = res.results[core]["xcout"]                   # [66, G, 64]
        mm = xc[0:64].reshape(8, 8, G, 8, 8)              # (he, de, g, j, h)
        num = np.einsum("hdgjh->hdgj", mm).reshape(64, npb)
        z4 = xc[64].reshape(G, 8, 8)
        b2e4 = xc[65].reshape(G, 8, 8)
        hh = np.arange(C) // D
        z = z4[:, :, hh].transpose(2, 0, 1).reshape(64, npb)
        b2e = b2e4[:, :, hh].transpose(2, 0, 1).reshape(64, npb)
        out[b, :, n0 : n0 + npb] = (num + wvs[:, None] * b2e) / z
    return out
